# revision 3
# baseline (speedup 1.0000x reference)
"""CountSketch TRN2 kernel: dense matmul in fp8e4 (DoubleRowSwInterleave) with
error compensation.

out[n, b*512+k] = sum_{d: i_hash[b,d]==k} x[n,d]*s_hash[b,d] / sqrt(B)
   == (x/sqrt(B)) @ P,  P[d, b*512+i_hash[b,d]] = s_hash[b,d] (+-1)

x is split as x ~= x1 + x2 with x1 = e4m3(x), x2 = e4m3(x - x1) (combined
relative error ~1.6e-3, same as bf16).  The stacked system [x1; x2] @ [P; P]
runs in fp8e4 with MatmulPerfMode.DoubleRowSwInterleave: each matmul
contracts 256 rows = two 128-d chunks (j, j+8) of one compensation stream,
so P is stored once.  Weights are host-interleaved in the SwInterleave
layout (A/B pairs per column, columns reversed).

Data-parallel over 8 NeuronCores: core i computes rows [i*1024, (i+1)*1024).
Per col-tile ct (32 of them): 16 DoubleRow matmuls accumulate psum
[128, 1024] f32 (split in two 512-col psum banks); DVE drains psum -> bf16
stage; ACT DMAs the stage to HBM.  outT is bf16 [4096, 1024] per core,
transposed + upcast to f32 on host.

Each rotating weight-slab slot has its own semaphore, so a wait can only be
satisfied when every SDMA engine has finished that exact transfer (the
previous kernel shared one counting semaphore across all input DMAs, which
let a straggler engine leave a weight tile incomplete while the count still
passed — intermittent wrong results on random cores).

Measured on trn2 (8 cores, Fori-loop reps differencing): ~221 us per
invocation vs ~255 us (f32r matmul formulation) for the previous kernel;
full-output rel err 2.39e-3, bitwise deterministic across runs.
"""
import numpy as np
import ml_dtypes
import concourse.bass as bass
from concourse import mybir
from concourse.bass_utils import run_bass_kernel_spmd

N_CORES = 8
N_FULL = 8192
D_IN = 2048
BLOCK_SIZE = 512
N_BLOCKS = 8
C_OUT = N_BLOCKS * BLOCK_SIZE      # 4096
M = N_FULL // N_CORES              # 1024 rows per core
CT = C_OUT // 128                  # 32 col-tiles
PAIRS = 8                          # 16 d-chunks paired (j, j+8)
SLOTS = 4

FP8 = mybir.dt.float8e4
NP_FP8 = ml_dtypes.float8_e4m3
PERF = mybir.MatmulPerfMode.DoubleRowSwInterleave


def build_nc(reps: int = 1) -> bass.Bass:
    """reps>1 wraps the body in a hardware Fori loop with a barrier +
    semaphore clear between reps (constant instruction count; used by the
    benchmark harness to resolve device time above dispatch noise)."""
    nc = bass.Bass(trn_type="TRN2", target_bir_lowering=False, debug=False)

    xt_d = nc.dram_tensor("xt", [128, 2 * 16 * M], FP8, kind="ExternalInput").ap()
    pt_d = nc.dram_tensor("pt", [CT, 128, PAIRS * 2 * 128], FP8,
                          kind="ExternalInput").ap()
    out_d = nc.dram_tensor("outT", [C_OUT, M], mybir.dt.bfloat16,
                           kind="ExternalOutput").ap()

    xt_sb = nc.alloc_sbuf_tensor("xt_sb", [128, 2 * 16 * M], FP8).ap()
    p_sb = [nc.alloc_sbuf_tensor(f"p_sb{s}", [128, PAIRS * 2 * 128], FP8).ap()
            for s in range(SLOTS)]
    stage = [nc.alloc_sbuf_tensor(f"stage{s}", [128, M], mybir.dt.bfloat16).ap()
             for s in range(SLOTS)]
    ps = [nc.alloc_psum_tensor(f"ps{s}", [128, M], mybir.dt.float32).ap()
          for s in range(SLOTS)]

    xt_v = xt_sb.tensor.ap().rearrange("p (c t n) -> p c t n", c=2, t=16, n=M)
    p_v = [p_sb[s].tensor.ap().rearrange("p (j s2 c) -> p j s2 c",
                                         j=PAIRS, s2=2, c=128)
           for s in range(SLOTS)]

    xt_sem = nc.alloc_semaphore("xt_sem")
    pt_sem = [nc.alloc_semaphore(f"pt_sem{s}") for s in range(SLOTS)]
    pe_sem = nc.alloc_semaphore("pe_sem")
    drain_sem = nc.alloc_semaphore("drain_sem")
    out_sem = nc.alloc_semaphore("out_sem")

    def body():
        for g in range(CT):
            s = g % SLOTS
            if g >= SLOTS:
                nc.sync.wait_ge(pe_sem, g - SLOTS + 1)
            nc.sync.dma_start(p_sb[s], pt_d[g]).then_inc(pt_sem[s], 16)

        for g in range(CT):
            s = g % SLOTS
            nc.tensor.wait_ge(pt_sem[s], 16 * (g // SLOTS + 1))
            if g >= SLOTS:
                nc.tensor.wait_ge(drain_sem, g - SLOTS + 1)
            mm = None
            for j in range(PAIRS):
                for c in range(2):
                    # mh innermost: one LDWEIGHTS serves both psum banks
                    for mh in range(2):
                        nh = slice(mh * 512, (mh + 1) * 512)
                        mm = nc.tensor.matmul(
                            ps[s][:, nh],
                            lhsT=p_v[s][:, j],
                            rhs=xt_v[:, c, j::PAIRS, nh],
                            start=(j == 0 and c == 0),
                            stop=(j == PAIRS - 1 and c == 1),
                            perf_mode=PERF,
                        )
            mm.then_inc(pe_sem, 1)

        for g in range(CT):
            s = g % SLOTS
            nc.vector.wait_ge(pe_sem, g + 1)
            if g >= SLOTS:
                nc.vector.wait_ge(out_sem, 16 * (g - SLOTS + 1))
            nc.vector.tensor_copy(stage[s], ps[s]).then_inc(drain_sem, 1)

        for g in range(CT):
            s = g % SLOTS
            nc.scalar.wait_ge(drain_sem, g + 1)
            nc.scalar.dma_start(out_d[g * 128:(g + 1) * 128, :],
                                stage[s]).then_inc(out_sem, 16)
        nc.scalar.wait_ge(out_sem, 16 * CT)

    nc.sync.dma_start(xt_sb, xt_d).then_inc(xt_sem, 16)
    nc.tensor.wait_ge(xt_sem, 16)

    if reps == 1:
        body()
    else:
        with nc.Fori(0, reps):
            body()
            nc.all_engine_barrier()
            for sem in (*pt_sem, pe_sem, drain_sem, out_sem):
                nc.sync.sem_clear(sem)
            nc.all_engine_barrier()

    return nc


def host_prep(x, s_hash, i_hash):
    xs = (np.asarray(x, np.float32) *
          np.float32(1.0 / np.sqrt(N_BLOCKS))).astype(np.float32)
    i_hash = np.asarray(i_hash)
    s_hash = np.asarray(s_hash, np.float32)

    # P[d, b*512 + i_hash[b,d]] = s_hash[b,d]
    P = np.zeros((D_IN, C_OUT), dtype=np.float32)
    d_idx = np.arange(D_IN)
    for b in range(N_BLOCKS):
        P[d_idx, b * BLOCK_SIZE + i_hash[b]] = s_hash[b]
    # logical pt[ct, p, j, s2, cc] = P[(j + 8*s2)*128 + p, ct*128 + cc]
    v = (P.reshape(2, PAIRS, 128, CT, 128)       # [s2, j, p, ct, cc]
          .transpose(3, 2, 1, 0, 4)              # [ct, p, j, s2, cc]
         ).astype(NP_FP8)
    # SwInterleave weight layout: per (ct, p, j) the 256 weights are stored
    # as pairs (A[127], B[127], A[126], B[126], ..., A[0], B[0]) where A/B
    # are the two k-subtiles and columns run in reverse order.
    A = v[..., 0, :][..., ::-1]
    B = v[..., 1, :][..., ::-1]
    sw = np.empty((CT, 128, PAIRS, 256), dtype=v.dtype)
    sw[..., 0::2] = A
    sw[..., 1::2] = B
    pt = np.ascontiguousarray(sw.reshape(CT, 128, PAIRS * 2 * 128))

    in_maps = []
    for i in range(N_CORES):
        shard_t = xs[i * M:(i + 1) * M].T        # [D, M]
        x1 = shard_t.astype(NP_FP8)
        x2 = (shard_t - x1.astype(np.float32)).astype(NP_FP8)
        xc = np.stack([x1, x2])                  # [comp, D, M]
        xt = np.ascontiguousarray(
            xc.reshape(2, 16, 128, M)            # [c, t, p, n]
              .transpose(2, 0, 1, 3)             # [p, c, t, n]
              .reshape(128, 2 * 16 * M)
        )
        in_maps.append({"xt": xt, "pt": pt})
    return in_maps


_NC_CACHE = {}


def kernel(x, s_hash, i_hash):
    if "nc" not in _NC_CACHE:
        _NC_CACHE["nc"] = build_nc(1)
    nc = _NC_CACHE["nc"]

    in_maps = host_prep(x, s_hash, i_hash)
    res = run_bass_kernel_spmd(nc, in_maps, list(range(N_CORES)), trace=False)

    out = np.empty((N_FULL, C_OUT), dtype=np.float32)
    for i in range(N_CORES):
        out[i * M:(i + 1) * M, :] = res.results[i]["outT"].astype(np.float32).T
    return out
